# revision 19
# baseline (speedup 1.0000x reference)
"""Trainium2 Bass kernel for CTGTernaryLinear.

Computes y = x @ w_eff.T where
  w_eff = sign(weight) * repeat16(softmax(pattern_logits) @ [1, .5, 0]) * scale

Sharding over 8 NeuronCores: DP=2 over tokens x TP=4 over output rows.
Per core: M=8192 tokens, N=1024 out-cols, K=4096 contraction.

Device pipeline per core:
  prep (per 128-wide k-chunk "ko"):
    exp(logits) on ScalarE -> expansion matmuls on PE (softmax-combine over
    the 3 pattern classes AND 16x block broadcast across partitions in one
    matmul with a constant basis matrix) -> reciprocal + sign-apply on
    VectorE -> w_effT tile resident in SBUF (fp32).
  GEMM: out[m-tile, n-chunk] accumulated over 32 k-chunks in PSUM,
    fp32r matmuls (full-rate FP32-in/FP22-internal), ScalarE copyback,
    DMA out.
"""

import numpy as np

import concourse.bacc as bacc
import concourse.mybir as mybir
import concourse.tile as tile
from concourse.bass_utils import run_bass_kernel_spmd

F32 = mybir.dt.float32
F32R = mybir.dt.float32r
ALU = mybir.AluOpType
ACTF = mybir.ActivationFunctionType

# Problem shapes (hardcoded per contract)
B, S, D_IN, D_OUT = 8, 2048, 4096, 4096
BLOCK = 16
M_TOT = B * S  # 16384
DP, TP = 2, 4
N_CORES = DP * TP
M_CORE = M_TOT // DP  # 8192
N_CORE = D_OUT // TP  # 1024
KO = D_IN // 128  # 32 k-chunks of 128
MT = M_CORE // 128  # 64 m-tiles
NH = N_CORE // 512  # 2 n-chunks of 512
JB = 128 // BLOCK  # 8 block-rows per k-chunk partition group


def build_nc(m_tiles=MT, n_core=N_CORE, matmul_dtype=F32R, loop_reps=1, variant='full5'):
    """Build the per-core Bass program. SPMD: same program all cores.

    loop_reps > 1 wraps the whole body in a hardware For_i loop (identical
    compute each iteration) — used only for wall-clock slope timing.
    """
    nh = n_core // 512
    nc = bacc.Bacc(None, target_bir_lowering=False, debug=False)
    MMDT = matmul_dtype

    # DRAM I/O (per-core layouts, host pre-arranged for contiguous DMA)
    x_t = nc.declare_dram_parameter("x_t", [128, m_tiles, KO, 128], MMDT, isOutput=False)
    w_t = nc.declare_dram_parameter("w_t", [128, KO, n_core], MMDT if variant.startswith("gemm") else F32, isOutput=False)
    pl_t = nc.declare_dram_parameter("pl_t", [3 * JB, KO, n_core], F32, isOutput=False)
    e_num = nc.declare_dram_parameter("e_num", [3 * JB, 128], MMDT, isOutput=False)
    e_den = nc.declare_dram_parameter("e_den", [3 * JB, 128], MMDT, isOutput=False)
    out = nc.declare_dram_parameter("out", [m_tiles, 128, n_core], F32, isOutput=True)

    with tile.TileContext(nc) as tc:
        with (
            tc.tile_pool(name="const", bufs=1) as const,
            tc.tile_pool(name="weff", bufs=1) as weffp,
            tc.tile_pool(name="prep", bufs=2) as prep,
            tc.tile_pool(name="ppsum", bufs=2, space="PSUM") as ppsum,
            tc.tile_pool(name="xin", bufs=2) as xin,
            tc.tile_pool(name="gpsum", bufs=4 if variant in ("full5", "full6") else 2, space="PSUM") as gpsum,
            tc.tile_pool(name="oout", bufs=2) as oout,
        ):
            en = const.tile([3 * JB, 128], MMDT)
            ed = const.tile([3 * JB, 128], MMDT)
            nc.sync.dma_start(out=en[:], in_=e_num[:])
            nc.sync.dma_start(out=ed[:], in_=e_den[:])

            if variant in ("full2", "full3", "full4", "full5", "full6"):
                w_eff = [
                    weffp.tile([128, n_core], MMDT, tag=f"weff{ko}", name=f"weff{ko}")
                    for ko in range(KO)
                ]
                wsl = lambda ko, sl: w_eff[ko][:, sl]
            else:
                w_eff_t = weffp.tile([128, KO, n_core], MMDT)
                w_eff = [w_eff_t[:, ko, :] for ko in range(KO)]
                wsl = lambda ko, sl: w_eff_t[:, ko, sl]

            def emit_body():
                if variant.startswith("full") or variant == "prep":
                    emit_prep()
                else:
                    nc.sync.dma_start(out=w_eff_t[:], in_=w_t[:])
                if variant != "prep":
                    emit_gemm()

            def emit_prep5():
                for ko in range(KO):
                    plc = prep.tile([3 * JB, n_core], F32, tag="plc")
                    nc.sync.dma_start(out=plc[:], in_=pl_t[:, ko, :])
                    expc = prep.tile([3 * JB, n_core], MMDT, tag="expc")
                    nc.scalar.activation(expc[:], plc[:], ACTF.Exp)
                    wc = prep.tile([128, n_core], F32, tag="wc")
                    nc.sync.dma_start(out=wc[:], in_=w_t[:, ko, :])
                    npp = ppsum.tile([128, nh, 512], F32, tag="npp", bufs=1)
                    dpp = ppsum.tile([128, nh, 512], F32, tag="dpp", bufs=1)
                    for h in range(nh):
                        sl = slice(h * 512, h * 512 + 512)
                        nc.tensor.matmul(npp[:, h, :], en[:], expc[:, sl])
                        nc.tensor.matmul(dpp[:, h, :], ed[:], expc[:, sl])
                    rec = prep.tile([128, nh, 512], F32, tag="rec", bufs=1)
                    nc.vector.reciprocal(rec[:], dpp[:])
                    mlt = prep.tile([128, nh, 512], F32, tag="mlt", bufs=1)
                    nc.vector.tensor_mul(mlt[:], npp[:], rec[:])
                    wc3 = wc[:].rearrange("p (h n) -> p h n", h=nh)
                    u = prep.tile([128, nh, 512], F32, tag="rec", bufs=1)
                    nc.vector.scalar_tensor_tensor(
                        u[:], wc3, 0.0, mlt[:], ALU.is_ge, ALU.mult
                    )
                    w3 = w_eff[ko][:].rearrange("p (h n) -> p h n", h=nh)
                    nc.vector.scalar_tensor_tensor(
                        w3, u[:], 2.0, mlt[:], ALU.mult, ALU.subtract
                    )

            def emit_prep():
                if variant in ("full5", "full6"):
                    emit_prep5()
                    return
                for ko in range(KO):
                    plc = prep.tile([3 * JB, n_core], F32, tag="plc")
                    nc.sync.dma_start(out=plc[:], in_=pl_t[:, ko, :])
                    expc = prep.tile([3 * JB, n_core], MMDT, tag="expc")
                    nc.scalar.activation(expc[:], plc[:], ACTF.Exp)
                    wc = prep.tile([128, n_core], F32, tag="wc")
                    nc.sync.dma_start(out=wc[:], in_=w_t[:, ko, :])
                    for h in range(nh):
                        sl = slice(h * 512, h * 512 + 512)
                        nps = ppsum.tile([128, 512], F32, tag="nps")
                        dps = ppsum.tile([128, 512], F32, tag="dps")
                        nc.tensor.matmul(nps[:], en[:], expc[:, sl])
                        nc.tensor.matmul(dps[:], ed[:], expc[:, sl])
                        rec = prep.tile([128, 512], F32, tag="rec")
                        nc.vector.reciprocal(rec[:], dps[:])
                        mlt = prep.tile([128, 512], F32, tag="mlt")
                        nc.vector.tensor_mul(mlt[:], nps[:], rec[:])
                        if variant == "full4":
                            # u on GpSimd (frees VectorE), final rounded op on DVE
                            u = prep.tile([128, 512], F32, tag="rec")
                            nc.gpsimd.scalar_tensor_tensor(
                                u[:], wc[:, sl], 0.0, mlt[:], ALU.is_ge, ALU.mult
                            )
                            nc.vector.scalar_tensor_tensor(
                                wsl(ko, sl), u[:], 2.0, mlt[:], ALU.mult, ALU.subtract
                            )
                        else:
                            # u = (w >= 0) * mlt ; w_eff = 2*u - mlt
                            u = prep.tile([128, 512], F32, tag="rec")
                            nc.vector.scalar_tensor_tensor(
                                u[:], wc[:, sl], 0.0, mlt[:], ALU.is_ge, ALU.mult
                            )
                            nc.vector.scalar_tensor_tensor(
                                wsl(ko, sl), u[:], 2.0, mlt[:], ALU.mult, ALU.subtract
                            )

            def emit_gemm():
                xt_shared = None
                if variant == "gemm_nodma":
                    xt_shared = xin.tile([128, KO, 128], MMDT, tag="xt")
                    nc.sync.dma_start(out=xt_shared[:], in_=x_t[:, 0, :, :])
                for mt in range(m_tiles):
                    if xt_shared is None:
                        xt = xin.tile([128, KO, 128], MMDT, tag="xt")
                        nc.sync.dma_start(out=xt[:], in_=x_t[:, mt, :, :])
                    else:
                        xt = xt_shared
                    ot = oout.tile([128, n_core], F32, tag="ot")
                    if variant in ("full3", "full6"):
                        pss = [gpsum.tile([128, 512], F32, tag="ps", name=f"ps{mt}_{i}") for i in range(nh)]
                        for ko in range(KO):
                            for h in range(nh):
                                nc.tensor.matmul(
                                    pss[h][:],
                                    xt[:, ko, :],
                                    wsl(ko, slice(h * 512, h * 512 + 512)),
                                    start=(ko == 0),
                                    stop=(ko == KO - 1),
                                )
                        for h in range(nh):
                            sl = slice(h * 512, h * 512 + 512)
                            nc.scalar.activation(ot[:, sl], pss[h][:], ACTF.Copy)
                    else:
                        for h in range(nh):
                            sl = slice(h * 512, h * 512 + 512)
                            ps = gpsum.tile([128, 512], F32, tag="ps")
                            for ko in range(KO):
                                nc.tensor.matmul(
                                    ps[:],
                                    xt[:, ko, :],
                                    wsl(ko, slice(h * 512, h * 512 + 512)),
                                    start=(ko == 0),
                                    stop=(ko == KO - 1),
                                )
                            nc.scalar.activation(ot[:, sl], ps[:], ACTF.Copy)
                    if variant != "gemm_nodma":
                        nc.sync.dma_start(out=out[mt], in_=ot[:])

            if loop_reps == 1:
                emit_body()
            else:
                with tc.For_i(0, loop_reps, 1):
                    emit_body()

    nc.finalize()
    return nc


def make_basis(scale: float):
    """E matrices [24, 128]: softmax-combine over r and 16x partition expand.

    Partition index (j*3 + r), j = block-row within a 128-k chunk, r = class.
    e_num[(j,r), kp] = (kp//16 == j) * [scale, scale/2, 0][r]
    e_den[(j,r), kp] = (kp//16 == j)
    """
    kp = np.arange(128)
    jmask = (kp[None, :] // BLOCK == np.arange(JB)[:, None]).astype(np.float32)
    coeff = np.array([1.0, 0.5, 0.0], dtype=np.float32) * np.float32(scale)
    e_num = (jmask[:, None, :] * coeff[None, :, None]).reshape(3 * JB, 128)
    e_den = np.repeat(jmask[:, None, :], 3, axis=1).reshape(3 * JB, 128)
    return np.ascontiguousarray(e_num), np.ascontiguousarray(e_den)


def make_in_maps(x, weight, pattern_logits, scale):
    """Host-side sharding + layout staging (pure data movement + scaling the
    3-element pattern basis by the scalar input)."""
    x2 = np.asarray(x, dtype=np.float32).reshape(M_TOT, D_IN)
    w = np.asarray(weight, dtype=np.float32)
    pl = np.asarray(pattern_logits, dtype=np.float32)
    e_num, e_den = make_basis(float(np.asarray(scale)))

    # x (per dp half): [M, K] -> [kp, mt, ko, ml]
    xts = []
    for dp in range(DP):
        xs = x2[dp * M_CORE : (dp + 1) * M_CORE]
        x4 = xs.reshape(MT, 128, KO, 128)  # [mt, ml, ko, kp]
        xts.append(np.ascontiguousarray(x4.transpose(3, 0, 2, 1)))

    wts, plts = [], []
    for tp in range(TP):
        ws = w[tp * N_CORE : (tp + 1) * N_CORE]  # [n, k]
        w3 = ws.reshape(N_CORE, KO, 128)  # [n, ko, kp]
        wts.append(np.ascontiguousarray(w3.transpose(2, 1, 0)))
        ps = pl[tp * N_CORE * (D_IN // BLOCK) : (tp + 1) * N_CORE * (D_IN // BLOCK)]
        # block index b = n*(D_IN//BLOCK) + ko*JB + j
        p4 = ps.reshape(N_CORE, KO, JB, 3)  # [n, ko, j, r]
        plts.append(np.ascontiguousarray(p4.transpose(2, 3, 1, 0).reshape(3 * JB, KO, N_CORE)))

    in_maps = []
    for c in range(N_CORES):
        dp, tp = divmod(c, TP)
        in_maps.append(
            {
                "x_t": xts[dp],
                "w_t": wts[tp],
                "pl_t": plts[tp],
                "e_num": e_num,
                "e_den": e_den,
            }
        )
    return in_maps


_NC_CACHE = {}


def get_nc():
    if "nc" not in _NC_CACHE:
        _NC_CACHE["nc"] = build_nc()
    return _NC_CACHE["nc"]


def kernel(x, weight, pattern_logits, scale):
    nc = get_nc()
    in_maps = make_in_maps(x, weight, pattern_logits, scale)
    res = run_bass_kernel_spmd(nc, in_maps, list(range(N_CORES)))
    y = np.empty((M_TOT, D_OUT), dtype=np.float32)
    for c in range(N_CORES):
        dp, tp = divmod(c, TP)
        o = res.results[c]["out"].reshape(M_CORE, N_CORE)
        y[dp * M_CORE : (dp + 1) * M_CORE, tp * N_CORE : (tp + 1) * N_CORE] = o
    return y.reshape(B, S, D_OUT)


# revision 24
# speedup vs baseline: 1.0220x; 1.0220x over previous
"""Trainium2 Bass kernel for CTGTernaryLinear.

Computes y = x @ w_eff.T where
  w_eff = sign(weight) * repeat16(softmax(pattern_logits) @ [1, .5, 0]) * scale

Sharding over 8 NeuronCores: DP=2 over tokens x TP=4 over output rows.
Per core: M=8192 tokens, N=1024 out-cols, K=4096 contraction.

Device pipeline per core:
  prep (per 128-wide k-chunk "ko"):
    exp(logits) on ScalarE -> expansion matmuls on PE (softmax-combine over
    the 3 pattern classes AND 16x block broadcast across partitions in one
    matmul with a constant basis matrix) -> reciprocal + sign-apply on
    VectorE -> w_effT tile resident in SBUF (fp32).
  GEMM: out[m-tile, n-chunk] accumulated over 32 k-chunks in PSUM,
    fp32r matmuls (full-rate FP32-in/FP22-internal), ScalarE copyback,
    DMA out.
"""

import numpy as np

import concourse.bacc as bacc
import concourse.mybir as mybir
import concourse.tile as tile
from concourse.bass_utils import run_bass_kernel_spmd

F32 = mybir.dt.float32
F32R = mybir.dt.float32r
ALU = mybir.AluOpType
ACTF = mybir.ActivationFunctionType

# Problem shapes (hardcoded per contract)
B, S, D_IN, D_OUT = 8, 2048, 4096, 4096
BLOCK = 16
M_TOT = B * S  # 16384
DP, TP = 2, 4
N_CORES = DP * TP
M_CORE = M_TOT // DP  # 8192
N_CORE = D_OUT // TP  # 1024
KO = D_IN // 128  # 32 k-chunks of 128
MT = M_CORE // 128  # 64 m-tiles
NH = N_CORE // 512  # 2 n-chunks of 512
JB = 128 // BLOCK  # 8 block-rows per k-chunk partition group


def build_nc(m_tiles=MT, n_core=N_CORE, matmul_dtype=F32R, loop_reps=1, variant='full9'):
    """Build the per-core Bass program. SPMD: same program all cores.

    loop_reps > 1 wraps the whole body in a hardware For_i loop (identical
    compute each iteration) — used only for wall-clock slope timing.
    """
    nh = n_core // 512
    nc = bacc.Bacc(None, target_bir_lowering=False, debug=False)
    MMDT = matmul_dtype

    # DRAM I/O (per-core layouts, host pre-arranged for contiguous DMA)
    x_t = nc.declare_dram_parameter("x_t", [128, m_tiles, KO, 128], MMDT, isOutput=False)
    w_t = nc.declare_dram_parameter("w_t", [128, KO, n_core], MMDT if variant.startswith("gemm") else F32, isOutput=False)
    pl_t = nc.declare_dram_parameter("pl_t", [3 * JB, KO, n_core], F32, isOutput=False)
    e_num = nc.declare_dram_parameter("e_num", [3 * JB, 128], MMDT, isOutput=False)
    e_den = nc.declare_dram_parameter("e_den", [3 * JB, 128], MMDT, isOutput=False)
    out = nc.declare_dram_parameter("out", [m_tiles, 128, n_core], F32, isOutput=True)

    with tile.TileContext(nc) as tc:
        with (
            tc.tile_pool(name="const", bufs=1) as const,
            tc.tile_pool(name="weff", bufs=1) as weffp,
            tc.tile_pool(name="prep", bufs=2) as prep,
            tc.tile_pool(name="ppsum", bufs=2, space="PSUM") as ppsum,
            tc.tile_pool(name="xin", bufs=2) as xin,
            tc.tile_pool(name="gpsum", bufs=4 if variant in ("full5", "full6", "full7", "full8", "full9") else 2, space="PSUM") as gpsum,
            tc.tile_pool(name="oout", bufs=2) as oout,
        ):
            en = const.tile([3 * JB, 128], MMDT)
            ed = const.tile([3 * JB, 128], MMDT)
            nc.sync.dma_start(out=en[:], in_=e_num[:])
            nc.sync.dma_start(out=ed[:], in_=e_den[:])

            if variant in ("full2", "full3", "full4", "full5", "full6", "full7", "full8", "full9"):
                w_eff = [
                    weffp.tile([128, n_core], MMDT, tag=f"weff{ko}", name=f"weff{ko}")
                    for ko in range(KO)
                ]
                wsl = lambda ko, sl: w_eff[ko][:, sl]
            else:
                w_eff_t = weffp.tile([128, KO, n_core], MMDT)
                w_eff = [w_eff_t[:, ko, :] for ko in range(KO)]
                wsl = lambda ko, sl: w_eff_t[:, ko, sl]

            def emit_body():
                if variant.startswith("full") or variant == "prep":
                    emit_prep()
                else:
                    nc.sync.dma_start(out=w_eff_t[:], in_=w_t[:])
                if variant != "prep":
                    emit_gemm()

            def emit_prep7():
                for ko in range(KO):
                    plc = prep.tile([3 * JB, n_core], F32, tag="plc")
                    nc.sync.dma_start(out=plc[:], in_=pl_t[:, ko, :])
                    expc = prep.tile([3 * JB, n_core], MMDT, tag="expc")
                    nc.scalar.activation(expc[:], plc[:], ACTF.Exp)
                    wc = prep.tile([128, n_core], F32, tag="wc")
                    nc.sync.dma_start(out=wc[:], in_=w_t[:, ko, :])
                    mlt = prep.tile([128, nh, 512], F32, tag="mlt", bufs=2 if variant == "full8" else 1)
                    for h in range(nh):
                        sl = slice(h * 512, h * 512 + 512)
                        nps = ppsum.tile([128, 512], F32, tag="nps")
                        dps = ppsum.tile([128, 512], F32, tag="dps")
                        nc.tensor.matmul(nps[:], en[:], expc[:, sl])
                        nc.tensor.matmul(dps[:], ed[:], expc[:, sl])
                        if variant == "full8":
                            nc.vector.tensor_tensor(mlt[:, h, :], nps[:], dps[:], ALU.divide)
                        else:
                            rec = prep.tile([128, 512], F32, tag="rec", bufs=2)
                            nc.vector.reciprocal(rec[:], dps[:])
                            nc.vector.tensor_mul(mlt[:, h, :], nps[:], rec[:])
                    wc3 = wc[:].rearrange("p (h n) -> p h n", h=nh)
                    u = prep.tile([128, nh, 512], F32, tag="u", bufs=1)
                    nc.vector.scalar_tensor_tensor(
                        u[:], wc3, 0.0, mlt[:], ALU.is_ge, ALU.mult
                    )
                    w3 = w_eff[ko][:].rearrange("p (h n) -> p h n", h=nh)
                    nc.vector.scalar_tensor_tensor(
                        w3, u[:], 2.0, mlt[:], ALU.mult, ALU.subtract
                    )

            def emit_prep5():
                for ko in range(KO):
                    plc = prep.tile([3 * JB, n_core], F32, tag="plc")
                    nc.sync.dma_start(out=plc[:], in_=pl_t[:, ko, :])
                    expc = prep.tile([3 * JB, n_core], MMDT, tag="expc")
                    nc.scalar.activation(expc[:], plc[:], ACTF.Exp)
                    wc = prep.tile([128, n_core], F32, tag="wc")
                    nc.sync.dma_start(out=wc[:], in_=w_t[:, ko, :])
                    npp = ppsum.tile([128, nh, 512], F32, tag="npp", bufs=1)
                    dpp = ppsum.tile([128, nh, 512], F32, tag="dpp", bufs=1)
                    for h in range(nh):
                        sl = slice(h * 512, h * 512 + 512)
                        nc.tensor.matmul(npp[:, h, :], en[:], expc[:, sl])
                        nc.tensor.matmul(dpp[:, h, :], ed[:], expc[:, sl])
                    rec = prep.tile([128, nh, 512], F32, tag="rec", bufs=1)
                    nc.vector.reciprocal(rec[:], dpp[:])
                    mlt = prep.tile([128, nh, 512], F32, tag="mlt", bufs=1)
                    nc.vector.tensor_mul(mlt[:], npp[:], rec[:])
                    wc3 = wc[:].rearrange("p (h n) -> p h n", h=nh)
                    u = prep.tile([128, nh, 512], F32, tag="rec", bufs=1)
                    nc.vector.scalar_tensor_tensor(
                        u[:], wc3, 0.0, mlt[:], ALU.is_ge, ALU.mult
                    )
                    w3 = w_eff[ko][:].rearrange("p (h n) -> p h n", h=nh)
                    nc.vector.scalar_tensor_tensor(
                        w3, u[:], 2.0, mlt[:], ALU.mult, ALU.subtract
                    )

            def emit_prep():
                if variant in ("full5", "full6"):
                    emit_prep5()
                    return
                if variant in ("full7", "full8", "full9"):
                    emit_prep7()
                    return
                for ko in range(KO):
                    plc = prep.tile([3 * JB, n_core], F32, tag="plc")
                    nc.sync.dma_start(out=plc[:], in_=pl_t[:, ko, :])
                    expc = prep.tile([3 * JB, n_core], MMDT, tag="expc")
                    nc.scalar.activation(expc[:], plc[:], ACTF.Exp)
                    wc = prep.tile([128, n_core], F32, tag="wc")
                    nc.sync.dma_start(out=wc[:], in_=w_t[:, ko, :])
                    for h in range(nh):
                        sl = slice(h * 512, h * 512 + 512)
                        nps = ppsum.tile([128, 512], F32, tag="nps")
                        dps = ppsum.tile([128, 512], F32, tag="dps")
                        nc.tensor.matmul(nps[:], en[:], expc[:, sl])
                        nc.tensor.matmul(dps[:], ed[:], expc[:, sl])
                        rec = prep.tile([128, 512], F32, tag="rec")
                        nc.vector.reciprocal(rec[:], dps[:])
                        mlt = prep.tile([128, 512], F32, tag="mlt")
                        nc.vector.tensor_mul(mlt[:], nps[:], rec[:])
                        if variant == "full4":
                            # u on GpSimd (frees VectorE), final rounded op on DVE
                            u = prep.tile([128, 512], F32, tag="rec")
                            nc.gpsimd.scalar_tensor_tensor(
                                u[:], wc[:, sl], 0.0, mlt[:], ALU.is_ge, ALU.mult
                            )
                            nc.vector.scalar_tensor_tensor(
                                wsl(ko, sl), u[:], 2.0, mlt[:], ALU.mult, ALU.subtract
                            )
                        else:
                            # u = (w >= 0) * mlt ; w_eff = 2*u - mlt
                            u = prep.tile([128, 512], F32, tag="rec")
                            nc.vector.scalar_tensor_tensor(
                                u[:], wc[:, sl], 0.0, mlt[:], ALU.is_ge, ALU.mult
                            )
                            nc.vector.scalar_tensor_tensor(
                                wsl(ko, sl), u[:], 2.0, mlt[:], ALU.mult, ALU.subtract
                            )

            def emit_gemm():
                xt_shared = None
                if variant == "gemm_nodma":
                    xt_shared = xin.tile([128, KO, 128], MMDT, tag="xt")
                    nc.sync.dma_start(out=xt_shared[:], in_=x_t[:, 0, :, :])
                for mt in range(m_tiles):
                    if xt_shared is None:
                        xt = xin.tile([128, KO, 128], MMDT, tag="xt")
                        nc.sync.dma_start(out=xt[:], in_=x_t[:, mt, :, :])
                    else:
                        xt = xt_shared
                    ot = oout.tile([128, n_core], F32, tag="ot")
                    if variant in ("full3", "full6"):
                        pss = [gpsum.tile([128, 512], F32, tag="ps", name=f"ps{mt}_{i}") for i in range(nh)]
                        for ko in range(KO):
                            for h in range(nh):
                                nc.tensor.matmul(
                                    pss[h][:],
                                    xt[:, ko, :],
                                    wsl(ko, slice(h * 512, h * 512 + 512)),
                                    start=(ko == 0),
                                    stop=(ko == KO - 1),
                                )
                        for h in range(nh):
                            sl = slice(h * 512, h * 512 + 512)
                            nc.scalar.activation(ot[:, sl], pss[h][:], ACTF.Copy)
                    else:
                        for h in range(nh):
                            sl = slice(h * 512, h * 512 + 512)
                            ps = gpsum.tile([128, 512], F32, tag="ps")
                            for ko in range(KO):
                                nc.tensor.matmul(
                                    ps[:],
                                    xt[:, ko, :],
                                    wsl(ko, slice(h * 512, h * 512 + 512)),
                                    start=(ko == 0),
                                    stop=(ko == KO - 1),
                                )
                            if variant == "full9":
                                nc.vector.tensor_copy(ot[:, sl], ps[:])
                            else:
                                nc.scalar.activation(ot[:, sl], ps[:], ACTF.Copy)
                    if variant != "gemm_nodma":
                        nc.sync.dma_start(out=out[mt], in_=ot[:])

            if loop_reps == 1:
                emit_body()
            else:
                with tc.For_i(0, loop_reps, 1):
                    emit_body()

    nc.finalize()
    return nc


def make_basis(scale: float):
    """E matrices [24, 128]: softmax-combine over r and 16x partition expand.

    Partition index (j*3 + r), j = block-row within a 128-k chunk, r = class.
    e_num[(j,r), kp] = (kp//16 == j) * [scale, scale/2, 0][r]
    e_den[(j,r), kp] = (kp//16 == j)
    """
    kp = np.arange(128)
    jmask = (kp[None, :] // BLOCK == np.arange(JB)[:, None]).astype(np.float32)
    coeff = np.array([1.0, 0.5, 0.0], dtype=np.float32) * np.float32(scale)
    e_num = (jmask[:, None, :] * coeff[None, :, None]).reshape(3 * JB, 128)
    e_den = np.repeat(jmask[:, None, :], 3, axis=1).reshape(3 * JB, 128)
    return np.ascontiguousarray(e_num), np.ascontiguousarray(e_den)


def make_in_maps(x, weight, pattern_logits, scale):
    """Host-side sharding + layout staging (pure data movement + scaling the
    3-element pattern basis by the scalar input)."""
    x2 = np.asarray(x, dtype=np.float32).reshape(M_TOT, D_IN)
    w = np.asarray(weight, dtype=np.float32)
    pl = np.asarray(pattern_logits, dtype=np.float32)
    e_num, e_den = make_basis(float(np.asarray(scale)))

    # x (per dp half): [M, K] -> [kp, mt, ko, ml]
    xts = []
    for dp in range(DP):
        xs = x2[dp * M_CORE : (dp + 1) * M_CORE]
        x4 = xs.reshape(MT, 128, KO, 128)  # [mt, ml, ko, kp]
        xts.append(np.ascontiguousarray(x4.transpose(3, 0, 2, 1)))

    wts, plts = [], []
    for tp in range(TP):
        ws = w[tp * N_CORE : (tp + 1) * N_CORE]  # [n, k]
        w3 = ws.reshape(N_CORE, KO, 128)  # [n, ko, kp]
        wts.append(np.ascontiguousarray(w3.transpose(2, 1, 0)))
        ps = pl[tp * N_CORE * (D_IN // BLOCK) : (tp + 1) * N_CORE * (D_IN // BLOCK)]
        # block index b = n*(D_IN//BLOCK) + ko*JB + j
        p4 = ps.reshape(N_CORE, KO, JB, 3)  # [n, ko, j, r]
        plts.append(np.ascontiguousarray(p4.transpose(2, 3, 1, 0).reshape(3 * JB, KO, N_CORE)))

    in_maps = []
    for c in range(N_CORES):
        dp, tp = divmod(c, TP)
        in_maps.append(
            {
                "x_t": xts[dp],
                "w_t": wts[tp],
                "pl_t": plts[tp],
                "e_num": e_num,
                "e_den": e_den,
            }
        )
    return in_maps




# ---- NEFF disk cache (keyed on BIR content hash) ----
# The compile hook recompiles identical BIR in every process (~2.5 min);
# cache the packaged NEFF so repeated kernel() calls are cheap.
def _install_neff_cache():
    try:
        import hashlib
        import os
        import shutil

        import concourse.bass_utils as _bu
        from concourse import bass2jax as _b2j

        if getattr(_bu, "_neff_cache_installed", False):
            return
        cache_dir = os.path.join(
            os.environ.get("HOME", "/tmp"), ".cache", "bass_neff_cache"
        )
        os.makedirs(cache_dir, exist_ok=True)
        orig = _bu.compile_bir_kernel

        def cached(ant_bir_str, compile_dir_path, neff_name="kernel.neff", **kw):
            try:
                key = hashlib.sha256(
                    ant_bir_str if isinstance(ant_bir_str, bytes) else ant_bir_str.encode()
                ).hexdigest()[:32]
                cpath = os.path.join(cache_dir, f"{key}_{neff_name}")
                dest = os.path.join(compile_dir_path, neff_name)
                if os.path.exists(cpath):
                    shutil.copyfile(cpath, dest)
                    return dest
                out = orig(ant_bir_str, compile_dir_path, neff_name=neff_name, **kw)
                try:
                    shutil.copyfile(out, cpath)
                except Exception:
                    pass
                return out
            except Exception:
                return orig(ant_bir_str, compile_dir_path, neff_name=neff_name, **kw)

        _bu.compile_bir_kernel = cached
        _bu._neff_cache_installed = True
        if getattr(_b2j, "compile_bir_kernel", None) is orig:
            _b2j.compile_bir_kernel = cached
    except Exception:
        pass


_install_neff_cache()


_NC_CACHE = {}


def get_nc():
    if "nc" not in _NC_CACHE:
        _NC_CACHE["nc"] = build_nc()
    return _NC_CACHE["nc"]


def kernel(x, weight, pattern_logits, scale):
    nc = get_nc()
    in_maps = make_in_maps(x, weight, pattern_logits, scale)
    res = run_bass_kernel_spmd(nc, in_maps, list(range(N_CORES)))
    y = np.empty((M_TOT, D_OUT), dtype=np.float32)
    for c in range(N_CORES):
        dp, tp = divmod(c, TP)
        o = res.results[c]["out"].reshape(M_CORE, N_CORE)
        y[dp * M_CORE : (dp + 1) * M_CORE, tp * N_CORE : (tp + 1) * N_CORE] = o
    return y.reshape(B, S, D_OUT)


# revision 25
# speedup vs baseline: 1.0833x; 1.0600x over previous
"""Trainium2 Bass kernel for CTGTernaryLinear.

Computes y = x @ w_eff.T where
  w_eff = sign(weight) * repeat16(softmax(pattern_logits) @ [1, .5, 0]) * scale

Sharding over 8 NeuronCores: DP=2 over tokens x TP=4 over output rows.
Per core: M=8192 tokens, N=1024 out-cols, K=4096 contraction.

Device pipeline per core:
  prep (per 128-wide k-chunk "ko"):
    exp(logits) on ScalarE -> expansion matmuls on PE (softmax-combine over
    the 3 pattern classes AND 16x block broadcast across partitions in one
    matmul with a constant basis matrix) -> reciprocal + sign-apply on
    VectorE -> w_effT tile resident in SBUF (fp32).
  GEMM: out[m-tile, n-chunk] accumulated over 32 k-chunks in PSUM,
    fp32r matmuls (full-rate FP32-in/FP22-internal), VectorE copyback,
    DMA out.

Measured (8 cores, axon trn2): ~1.26-1.31 ms HW exec, rms rel err 1.57e-4
vs the f32 reference (fp32r FP22 input truncation).
"""

import numpy as np

import concourse.bacc as bacc
import concourse.mybir as mybir
import concourse.tile as tile
from concourse.bass_utils import run_bass_kernel_spmd

F32 = mybir.dt.float32
F32R = mybir.dt.float32r
ALU = mybir.AluOpType
ACTF = mybir.ActivationFunctionType

# Problem shapes (hardcoded per contract)
B, S, D_IN, D_OUT = 8, 2048, 4096, 4096
BLOCK = 16
M_TOT = B * S  # 16384
DP, TP = 2, 4
N_CORES = DP * TP
M_CORE = M_TOT // DP  # 8192
N_CORE = D_OUT // TP  # 1024
KO = D_IN // 128  # 32 k-chunks of 128
MT = M_CORE // 128  # 64 m-tiles
NH = N_CORE // 512  # 2 n-chunks of 512
JB = 128 // BLOCK  # 8 block-rows per k-chunk partition group


def build_nc(m_tiles=MT, n_core=N_CORE, matmul_dtype=F32R, loop_reps=1, variant='full9'):
    """Build the per-core Bass program. SPMD: same program all cores.

    loop_reps > 1 wraps the whole body in a hardware For_i loop (identical
    compute each iteration) — used only for wall-clock slope timing.
    """
    nh = n_core // 512
    nc = bacc.Bacc(None, target_bir_lowering=False, debug=False)
    MMDT = matmul_dtype

    # DRAM I/O (per-core layouts, host pre-arranged for contiguous DMA)
    x_t = nc.declare_dram_parameter("x_t", [128, m_tiles, KO, 128], MMDT, isOutput=False)
    w_t = nc.declare_dram_parameter("w_t", [128, KO, n_core], MMDT if variant.startswith("gemm") else F32, isOutput=False)
    pl_t = nc.declare_dram_parameter("pl_t", [3 * JB, KO, n_core], F32, isOutput=False)
    e_num = nc.declare_dram_parameter("e_num", [3 * JB, 128], MMDT, isOutput=False)
    e_den = nc.declare_dram_parameter("e_den", [3 * JB, 128], MMDT, isOutput=False)
    out = nc.declare_dram_parameter("out", [m_tiles, 128, n_core], F32, isOutput=True)

    with tile.TileContext(nc) as tc:
        with (
            tc.tile_pool(name="const", bufs=1) as const,
            tc.tile_pool(name="weff", bufs=1) as weffp,
            tc.tile_pool(name="prep", bufs=2) as prep,
            tc.tile_pool(name="ppsum", bufs=2, space="PSUM") as ppsum,
            tc.tile_pool(name="xin", bufs=2) as xin,
            tc.tile_pool(name="gpsum", bufs=4 if variant in ("full5", "full6", "full7", "full8", "full9") else 2, space="PSUM") as gpsum,
            tc.tile_pool(name="oout", bufs=2) as oout,
        ):
            en = const.tile([3 * JB, 128], MMDT)
            ed = const.tile([3 * JB, 128], MMDT)
            nc.sync.dma_start(out=en[:], in_=e_num[:])
            nc.sync.dma_start(out=ed[:], in_=e_den[:])

            if variant in ("full2", "full3", "full4", "full5", "full6", "full7", "full8", "full9"):
                w_eff = [
                    weffp.tile([128, n_core], MMDT, tag=f"weff{ko}", name=f"weff{ko}")
                    for ko in range(KO)
                ]
                wsl = lambda ko, sl: w_eff[ko][:, sl]
            else:
                w_eff_t = weffp.tile([128, KO, n_core], MMDT)
                w_eff = [w_eff_t[:, ko, :] for ko in range(KO)]
                wsl = lambda ko, sl: w_eff_t[:, ko, sl]

            def emit_body():
                if variant.startswith("full") or variant == "prep":
                    emit_prep()
                else:
                    nc.sync.dma_start(out=w_eff_t[:], in_=w_t[:])
                if variant != "prep":
                    emit_gemm()

            def emit_prep7():
                for ko in range(KO):
                    plc = prep.tile([3 * JB, n_core], F32, tag="plc")
                    nc.sync.dma_start(out=plc[:], in_=pl_t[:, ko, :])
                    expc = prep.tile([3 * JB, n_core], MMDT, tag="expc")
                    nc.scalar.activation(expc[:], plc[:], ACTF.Exp)
                    wc = prep.tile([128, n_core], F32, tag="wc")
                    nc.sync.dma_start(out=wc[:], in_=w_t[:, ko, :])
                    mlt = prep.tile([128, nh, 512], F32, tag="mlt", bufs=2 if variant == "full8" else 1)
                    for h in range(nh):
                        sl = slice(h * 512, h * 512 + 512)
                        nps = ppsum.tile([128, 512], F32, tag="nps")
                        dps = ppsum.tile([128, 512], F32, tag="dps")
                        nc.tensor.matmul(nps[:], en[:], expc[:, sl])
                        nc.tensor.matmul(dps[:], ed[:], expc[:, sl])
                        if variant == "full8":
                            nc.vector.tensor_tensor(mlt[:, h, :], nps[:], dps[:], ALU.divide)
                        else:
                            rec = prep.tile([128, 512], F32, tag="rec", bufs=2)
                            nc.vector.reciprocal(rec[:], dps[:])
                            nc.vector.tensor_mul(mlt[:, h, :], nps[:], rec[:])
                    wc3 = wc[:].rearrange("p (h n) -> p h n", h=nh)
                    u = prep.tile([128, nh, 512], F32, tag="u", bufs=1)
                    nc.vector.scalar_tensor_tensor(
                        u[:], wc3, 0.0, mlt[:], ALU.is_ge, ALU.mult
                    )
                    w3 = w_eff[ko][:].rearrange("p (h n) -> p h n", h=nh)
                    nc.vector.scalar_tensor_tensor(
                        w3, u[:], 2.0, mlt[:], ALU.mult, ALU.subtract
                    )

            def emit_prep5():
                for ko in range(KO):
                    plc = prep.tile([3 * JB, n_core], F32, tag="plc")
                    nc.sync.dma_start(out=plc[:], in_=pl_t[:, ko, :])
                    expc = prep.tile([3 * JB, n_core], MMDT, tag="expc")
                    nc.scalar.activation(expc[:], plc[:], ACTF.Exp)
                    wc = prep.tile([128, n_core], F32, tag="wc")
                    nc.sync.dma_start(out=wc[:], in_=w_t[:, ko, :])
                    npp = ppsum.tile([128, nh, 512], F32, tag="npp", bufs=1)
                    dpp = ppsum.tile([128, nh, 512], F32, tag="dpp", bufs=1)
                    for h in range(nh):
                        sl = slice(h * 512, h * 512 + 512)
                        nc.tensor.matmul(npp[:, h, :], en[:], expc[:, sl])
                        nc.tensor.matmul(dpp[:, h, :], ed[:], expc[:, sl])
                    rec = prep.tile([128, nh, 512], F32, tag="rec", bufs=1)
                    nc.vector.reciprocal(rec[:], dpp[:])
                    mlt = prep.tile([128, nh, 512], F32, tag="mlt", bufs=1)
                    nc.vector.tensor_mul(mlt[:], npp[:], rec[:])
                    wc3 = wc[:].rearrange("p (h n) -> p h n", h=nh)
                    u = prep.tile([128, nh, 512], F32, tag="rec", bufs=1)
                    nc.vector.scalar_tensor_tensor(
                        u[:], wc3, 0.0, mlt[:], ALU.is_ge, ALU.mult
                    )
                    w3 = w_eff[ko][:].rearrange("p (h n) -> p h n", h=nh)
                    nc.vector.scalar_tensor_tensor(
                        w3, u[:], 2.0, mlt[:], ALU.mult, ALU.subtract
                    )

            def emit_prep():
                if variant in ("full5", "full6"):
                    emit_prep5()
                    return
                if variant in ("full7", "full8", "full9"):
                    emit_prep7()
                    return
                for ko in range(KO):
                    plc = prep.tile([3 * JB, n_core], F32, tag="plc")
                    nc.sync.dma_start(out=plc[:], in_=pl_t[:, ko, :])
                    expc = prep.tile([3 * JB, n_core], MMDT, tag="expc")
                    nc.scalar.activation(expc[:], plc[:], ACTF.Exp)
                    wc = prep.tile([128, n_core], F32, tag="wc")
                    nc.sync.dma_start(out=wc[:], in_=w_t[:, ko, :])
                    for h in range(nh):
                        sl = slice(h * 512, h * 512 + 512)
                        nps = ppsum.tile([128, 512], F32, tag="nps")
                        dps = ppsum.tile([128, 512], F32, tag="dps")
                        nc.tensor.matmul(nps[:], en[:], expc[:, sl])
                        nc.tensor.matmul(dps[:], ed[:], expc[:, sl])
                        rec = prep.tile([128, 512], F32, tag="rec")
                        nc.vector.reciprocal(rec[:], dps[:])
                        mlt = prep.tile([128, 512], F32, tag="mlt")
                        nc.vector.tensor_mul(mlt[:], nps[:], rec[:])
                        if variant == "full4":
                            # u on GpSimd (frees VectorE), final rounded op on DVE
                            u = prep.tile([128, 512], F32, tag="rec")
                            nc.gpsimd.scalar_tensor_tensor(
                                u[:], wc[:, sl], 0.0, mlt[:], ALU.is_ge, ALU.mult
                            )
                            nc.vector.scalar_tensor_tensor(
                                wsl(ko, sl), u[:], 2.0, mlt[:], ALU.mult, ALU.subtract
                            )
                        else:
                            # u = (w >= 0) * mlt ; w_eff = 2*u - mlt
                            u = prep.tile([128, 512], F32, tag="rec")
                            nc.vector.scalar_tensor_tensor(
                                u[:], wc[:, sl], 0.0, mlt[:], ALU.is_ge, ALU.mult
                            )
                            nc.vector.scalar_tensor_tensor(
                                wsl(ko, sl), u[:], 2.0, mlt[:], ALU.mult, ALU.subtract
                            )

            def emit_gemm():
                xt_shared = None
                if variant == "gemm_nodma":
                    xt_shared = xin.tile([128, KO, 128], MMDT, tag="xt")
                    nc.sync.dma_start(out=xt_shared[:], in_=x_t[:, 0, :, :])
                for mt in range(m_tiles):
                    if xt_shared is None:
                        xt = xin.tile([128, KO, 128], MMDT, tag="xt")
                        nc.sync.dma_start(out=xt[:], in_=x_t[:, mt, :, :])
                    else:
                        xt = xt_shared
                    ot = oout.tile([128, n_core], F32, tag="ot")
                    if variant in ("full3", "full6"):
                        pss = [gpsum.tile([128, 512], F32, tag="ps", name=f"ps{mt}_{i}") for i in range(nh)]
                        for ko in range(KO):
                            for h in range(nh):
                                nc.tensor.matmul(
                                    pss[h][:],
                                    xt[:, ko, :],
                                    wsl(ko, slice(h * 512, h * 512 + 512)),
                                    start=(ko == 0),
                                    stop=(ko == KO - 1),
                                )
                        for h in range(nh):
                            sl = slice(h * 512, h * 512 + 512)
                            nc.scalar.activation(ot[:, sl], pss[h][:], ACTF.Copy)
                    else:
                        for h in range(nh):
                            sl = slice(h * 512, h * 512 + 512)
                            ps = gpsum.tile([128, 512], F32, tag="ps")
                            for ko in range(KO):
                                nc.tensor.matmul(
                                    ps[:],
                                    xt[:, ko, :],
                                    wsl(ko, slice(h * 512, h * 512 + 512)),
                                    start=(ko == 0),
                                    stop=(ko == KO - 1),
                                )
                            if variant == "full9":
                                nc.vector.tensor_copy(ot[:, sl], ps[:])
                            else:
                                nc.scalar.activation(ot[:, sl], ps[:], ACTF.Copy)
                    if variant != "gemm_nodma":
                        nc.sync.dma_start(out=out[mt], in_=ot[:])

            if loop_reps == 1:
                emit_body()
            else:
                with tc.For_i(0, loop_reps, 1):
                    emit_body()

    nc.finalize()
    return nc


def make_basis(scale: float):
    """E matrices [24, 128]: softmax-combine over r and 16x partition expand.

    Partition index (j*3 + r), j = block-row within a 128-k chunk, r = class.
    e_num[(j,r), kp] = (kp//16 == j) * [scale, scale/2, 0][r]
    e_den[(j,r), kp] = (kp//16 == j)
    """
    kp = np.arange(128)
    jmask = (kp[None, :] // BLOCK == np.arange(JB)[:, None]).astype(np.float32)
    coeff = np.array([1.0, 0.5, 0.0], dtype=np.float32) * np.float32(scale)
    e_num = (jmask[:, None, :] * coeff[None, :, None]).reshape(3 * JB, 128)
    e_den = np.repeat(jmask[:, None, :], 3, axis=1).reshape(3 * JB, 128)
    return np.ascontiguousarray(e_num), np.ascontiguousarray(e_den)


def make_in_maps(x, weight, pattern_logits, scale):
    """Host-side sharding + layout staging (pure data movement + scaling the
    3-element pattern basis by the scalar input)."""
    x2 = np.asarray(x, dtype=np.float32).reshape(M_TOT, D_IN)
    w = np.asarray(weight, dtype=np.float32)
    pl = np.asarray(pattern_logits, dtype=np.float32)
    e_num, e_den = make_basis(float(np.asarray(scale)))

    # x (per dp half): [M, K] -> [kp, mt, ko, ml]
    xts = []
    for dp in range(DP):
        xs = x2[dp * M_CORE : (dp + 1) * M_CORE]
        x4 = xs.reshape(MT, 128, KO, 128)  # [mt, ml, ko, kp]
        xts.append(np.ascontiguousarray(x4.transpose(3, 0, 2, 1)))

    wts, plts = [], []
    for tp in range(TP):
        ws = w[tp * N_CORE : (tp + 1) * N_CORE]  # [n, k]
        w3 = ws.reshape(N_CORE, KO, 128)  # [n, ko, kp]
        wts.append(np.ascontiguousarray(w3.transpose(2, 1, 0)))
        ps = pl[tp * N_CORE * (D_IN // BLOCK) : (tp + 1) * N_CORE * (D_IN // BLOCK)]
        # block index b = n*(D_IN//BLOCK) + ko*JB + j
        p4 = ps.reshape(N_CORE, KO, JB, 3)  # [n, ko, j, r]
        plts.append(np.ascontiguousarray(p4.transpose(2, 3, 1, 0).reshape(3 * JB, KO, N_CORE)))

    in_maps = []
    for c in range(N_CORES):
        dp, tp = divmod(c, TP)
        in_maps.append(
            {
                "x_t": xts[dp],
                "w_t": wts[tp],
                "pl_t": plts[tp],
                "e_num": e_num,
                "e_den": e_den,
            }
        )
    return in_maps




# ---- NEFF disk cache (keyed on BIR content hash) ----
# The compile hook recompiles identical BIR in every process (~2.5 min);
# cache the packaged NEFF so repeated kernel() calls are cheap.
def _install_neff_cache():
    try:
        import hashlib
        import os
        import shutil

        import concourse.bass_utils as _bu
        from concourse import bass2jax as _b2j

        if getattr(_bu, "_neff_cache_installed", False):
            return
        cache_dir = os.path.join(
            os.environ.get("HOME", "/tmp"), ".cache", "bass_neff_cache"
        )
        os.makedirs(cache_dir, exist_ok=True)
        orig = _bu.compile_bir_kernel

        def cached(ant_bir_str, compile_dir_path, neff_name="kernel.neff", **kw):
            try:
                key = hashlib.sha256(
                    ant_bir_str if isinstance(ant_bir_str, bytes) else ant_bir_str.encode()
                ).hexdigest()[:32]
                cpath = os.path.join(cache_dir, f"{key}_{neff_name}")
                dest = os.path.join(compile_dir_path, neff_name)
                if os.path.exists(cpath):
                    shutil.copyfile(cpath, dest)
                    return dest
                out = orig(ant_bir_str, compile_dir_path, neff_name=neff_name, **kw)
                try:
                    shutil.copyfile(out, cpath)
                except Exception:
                    pass
                return out
            except Exception:
                return orig(ant_bir_str, compile_dir_path, neff_name=neff_name, **kw)

        _bu.compile_bir_kernel = cached
        _bu._neff_cache_installed = True
        if getattr(_b2j, "compile_bir_kernel", None) is orig:
            _b2j.compile_bir_kernel = cached
    except Exception:
        pass


_install_neff_cache()


_NC_CACHE = {}


def get_nc():
    if "nc" not in _NC_CACHE:
        _NC_CACHE["nc"] = build_nc()
    return _NC_CACHE["nc"]


def kernel(x, weight, pattern_logits, scale):
    nc = get_nc()
    in_maps = make_in_maps(x, weight, pattern_logits, scale)
    res = run_bass_kernel_spmd(nc, in_maps, list(range(N_CORES)))
    y = np.empty((M_TOT, D_OUT), dtype=np.float32)
    for c in range(N_CORES):
        dp, tp = divmod(c, TP)
        o = res.results[c]["out"].reshape(M_CORE, N_CORE)
        y[dp * M_CORE : (dp + 1) * M_CORE, tp * N_CORE : (tp + 1) * N_CORE] = o
    return y.reshape(B, S, D_OUT)
